# revision 1
# baseline (speedup 1.0000x reference)
"""Trainium2 Bass kernel for conv-qkv rank-1 attention.

out = gamma * q * sum(k*v) + x, where q,k,v are per-time-slice 3x3 convs
(C=64 -> C=64) of x [B=8, C=64, T=16, W=64, H=64].

Sharding: data-parallel over B across 8 cores (1 example/core), conv
weights replicated. No cross-core communication.

Per-core schedule: T slices processed in pairs; slice t lives on SBUF
partitions 0-63, slice t+1 on partitions 64-127, giving two concurrent
PE row-group chains (K=64 each). Each 3x3 conv = 9 shifted matmul taps
(+1 bias tap with an all-ones rhs) accumulated in PSUM. Stationary
[Wq|Wk] (M=128) produces q,k in one bank; Wv (M=64) is column-placed so
k and v land on the same partitions for the fused DVE k*v+reduce.
Matmuls run in float32r (FP22 truncation on read, 1 cycle/row).
"""

import numpy as np

import concourse.bacc as bacc
import concourse.bass as bass
import concourse.mybir as mybir
import concourse.tile as tile
from concourse import bass_utils

F32 = mybir.dt.float32
F32R = mybir.dt.float32r
ALU = mybir.AluOpType

B, C, T, W, H = 8, 64, 16, 64, 64
WP, HP = W + 2, H + 2          # padded slice dims
import os
NPAIR = int(os.environ.get("BASS_NPAIR", T // 2))  # slice pairs per core
RB = 8                         # W-rows per pixel block
NBLK = W // RB                 # pixel blocks per slice
BN = RB * H                    # moving free dim per matmul (512)
NTAP = 10                      # 9 conv taps + 1 bias tap


def _round22(a: np.ndarray) -> np.ndarray:
    """Round fp32 to 11 mantissa bits so the PE's FP22 read-truncation is
    exact (unbiased quantization instead of truncation)."""
    u = np.ascontiguousarray(a, np.float32).view(np.uint32).astype(np.uint64)
    u = ((u + 0x800) & 0xFFFFF000).astype(np.uint32)
    return u.view(np.float32)


def _pack_weights(wq, wk, wv, bq, bk, bv):
    """Pack stationary operands.

    wqk [128, 10, 128]: partitions 0-63 = chain-low taps ([Wq | Wk] so q
    lands on psum partitions 0-63, matching x_t's partitions), partitions
    64-127 = chain-high taps ([Wk | Wq], q on partitions 64-127). Tap 9 is
    the bias tap (row 0 = biases, used with an all-ones rhs).
    wv2 [128, 10, 64]: Wv taps for both chains (same values).
    """
    def taps(w):  # [O, I, 1, 3, 3] -> [I, 9, O]
        return np.ascontiguousarray(
            w.reshape(C, C, 9).transpose(1, 2, 0), np.float32)

    wq_t, wk_t, wv_t = taps(wq), taps(wk), taps(wv)
    # [Wk | Wq] for both chains: k lands on psum partitions 0-63 (the
    # custom DVE reduce op requires base partition 0), q on 64-127
    wqk = np.zeros((128, NTAP, 128), np.float32)
    wqk[0:64, 0:9, 0:64] = wk_t
    wqk[0:64, 0:9, 64:128] = wq_t
    wqk[64:128, 0:9, 0:64] = wk_t
    wqk[64:128, 0:9, 64:128] = wq_t
    wqk[0, 9, 0:64] = bk
    wqk[0, 9, 64:128] = bq
    wqk[64, 9, 0:64] = bk
    wqk[64, 9, 64:128] = bq

    # v stationary is [Wv | Wv] (M=128): the duplicated column half costs
    # nothing (M=64 would leave the array half idle) and lets every matmul
    # use column position 0, which fp32r codegen requires
    wv2 = np.zeros((128, NTAP, 128), np.float32)
    wv2[0:64, 0:9, 0:64] = wv_t
    wv2[0:64, 0:9, 64:128] = wv_t
    wv2[64:128, 0:9, 0:64] = wv_t
    wv2[64:128, 0:9, 64:128] = wv_t
    wv2[0, 9, 0:64] = bv
    wv2[0, 9, 64:128] = bv
    wv2[64, 9, 0:64] = bv
    wv2[64, 9, 64:128] = bv
    return _round22(wqk), _round22(wv2)


def _emit(nc, tc, x_d, wqk_d, wv_d, gam_d, ones_d, zer_d, out_d, ctx):
    const = ctx.enter_context(tc.tile_pool(name="const", bufs=1))
    state = ctx.enter_context(tc.tile_pool(name="state", bufs=1))
    psum = ctx.enter_context(
        tc.tile_pool(name="psum", bufs=2, space=bass.MemorySpace.PSUM))
    vpool = ctx.enter_context(tc.tile_pool(name="vpool", bufs=2))

    wqk_t = const.tile([128, NTAP, 128], F32R, tag="wqk")
    wv_t = const.tile([128, NTAP, 128], F32R, tag="wv")
    gam_t = const.tile([128, 1], F32, tag="gam")
    ones_t = const.tile([128, BN], F32R, tag="ones")

    nc.sync.dma_start(wqk_t[:], wqk_d[:])
    nc.sync.dma_start(wv_t[:], wv_d[:])
    nc.sync.dma_start(gam_t[:], gam_d[:])
    nc.sync.dma_start(ones_t[:], ones_d[:])

    xp = [state.tile([128, WP, HP], F32R, tag=f"xp{i}", name=f"xp{i}") for i in range(3)]
    qs = [state.tile([128, W * H], F32, tag=f"qs{i}", name=f"qs{i}") for i in range(2)]
    ot = [state.tile([128, W * H], F32, tag=f"ot{i}", name=f"ot{i}") for i in range(2)]
    scr = state.tile([128, BN], F32, tag="scr")
    sparts = [state.tile([64, 2, NBLK], F32, tag=f"sp{i}", name=f"sp{i}") for i in range(2)]
    sgam = [state.tile([64, 2], F32, tag=f"sg{i}", name=f"sg{i}") for i in range(2)]
    sfin = [state.tile([128, 1], F32, tag=f"sf{i}", name=f"sf{i}") for i in range(2)]

    # zero the padding ring of both x buffers once (gpsimd memset does not
    # take f32r, so DMA from a host-provided zero vector); interior DMAs
    # never touch the ring
    for t_ in xp:
        nc.sync.dma_start(t_[:, 0, :], zer_d[:, :])
        nc.sync.dma_start(t_[:, WP - 1, :], zer_d[:, :])
        nc.sync.dma_start(t_[:, :, 0], zer_d[:, 0:WP])
        nc.sync.dma_start(t_[:, :, HP - 1], zer_d[:, 0:WP])

    def load_pair(p):
        t_ = xp[p % 3]
        nc.sync.dma_start(t_[0:64, 1:1 + W, 1:1 + H], x_d[:, 2 * p])
        nc.sync.dma_start(t_[64:128, 1:1 + W, 1:1 + H], x_d[:, 2 * p + 1])

    load_pair(0)
    if NPAIR > 1:
        load_pair(1)

    for p in range(NPAIR):
        pb = p % 2
        xp_, qs_, ot_ = xp[p % 3], qs[pb], ot[pb]

        # prefetch two pairs ahead; emitted before this pair's s-swap DMA
        # so the serial sync queue never holds the x-load behind it
        if p + 2 < NPAIR:
            load_pair(p + 2)

        for j in range(NBLK):
            qk_lo = psum.tile([128, BN], F32, tag="qk_lo")
            qk_hi = psum.tile([128, BN], F32, tag="qk_hi")
            v_lo = psum.tile([128, BN], F32, tag="v_lo", name="v_lo")
            v_hi = psum.tile([128, BN], F32, tag="v_hi", name="v_hi")

            def rhs(half, tap):
                if tap == 9:
                    return ones_t[64 * half:64 * half + 64, :]
                dy, dx = tap // 3, tap % 3
                r0 = j * RB + dy
                return xp_[64 * half:64 * half + 64,
                           r0:r0 + RB, dx:dx + H]

            for tap in range(NTAP):
                st, sp = tap == 0, tap == NTAP - 1
                nc.tensor.matmul(
                    qk_lo[:, :],
                    wqk_t[0:64, tap, :],
                    rhs(0, tap), start=st, stop=sp)
                nc.tensor.matmul(
                    qk_hi[:, :],
                    wqk_t[64:128, tap, :],
                    rhs(1, tap), start=st, stop=sp)
            for tap in range(NTAP):
                st, sp = tap == 0, tap == NTAP - 1
                nc.tensor.matmul(
                    v_lo[:, :],
                    wv_t[0:64, tap, :],
                    rhs(0, tap), start=st, stop=sp)
                nc.tensor.matmul(
                    v_hi[:, :],
                    wv_t[64:128, tap, :],
                    rhs(1, tap), start=st, stop=sp)

            # evacuate q and v on ScalarE (DVE may read only one PSUM
            # operand, so v must reach SBUF before the fused k*v reduce).
            # q_t moves partitions 64-127 -> 0-63 to line up with x_t.
            if os.environ.get("BASS_QCROSS", "1") == "1":
                nc.scalar.copy(qs_[0:64, j * BN:(j + 1) * BN], qk_lo[64:128, :])
            else:
                nc.scalar.copy(qs_[0:64, j * BN:(j + 1) * BN], qk_lo[0:64, :])
            nc.scalar.copy(qs_[64:128, j * BN:(j + 1) * BN], qk_hi[64:128, :])
            vsb_lo = vpool.tile([64, BN], F32, tag="vsb_lo", name="vsb_lo")
            vsb_hi = vpool.tile([64, BN], F32, tag="vsb_hi", name="vsb_hi")
            nc.scalar.copy(vsb_lo[:, :], v_lo[0:64, :])
            nc.scalar.copy(vsb_hi[:, :], v_hi[0:64, :])

            # fused k*v multiply + pixel-sum (k from PSUM at base partition
            # 0 -- the custom DVE op requires it; v from SBUF)
            if os.environ.get("BASS_TTR", "1") == "1":
                # native TensorScalarPtr with accumulate: one DVE pass does
                # k*v and the pixel-sum
                nc.vector.scalar_tensor_tensor(
                    out=scr[0:64, :], in0=qk_lo[0:64, :], scalar=1.0,
                    in1=vsb_lo[:, :], op0=ALU.mult, op1=ALU.mult,
                    accum_out=sparts[pb][:, 0, j:j + 1])
                nc.vector.scalar_tensor_tensor(
                    out=scr[0:64, :], in0=qk_hi[0:64, :], scalar=1.0,
                    in1=vsb_hi[:, :], op0=ALU.mult, op1=ALU.mult,
                    accum_out=sparts[pb][:, 1, j:j + 1])
            else:
                nc.vector.tensor_tensor(
                    out=scr[0:64, :], in0=qk_lo[0:64, :], in1=vsb_lo[:, :],
                    op=ALU.mult)
                nc.vector.reduce_sum(sparts[pb][:, 0, j:j + 1], scr[0:64, :],
                                     axis=mybir.AxisListType.X)
                nc.vector.tensor_tensor(
                    out=scr[0:64, :], in0=qk_hi[0:64, :], in1=vsb_hi[:, :],
                    op=ALU.mult)
                nc.vector.reduce_sum(sparts[pb][:, 1, j:j + 1], scr[0:64, :],
                                     axis=mybir.AxisListType.X)

        nc.vector.reduce_sum(sgam[pb][:, :], sparts[pb][:, :, :],
                             axis=mybir.AxisListType.X)
        nc.vector.tensor_scalar_mul(sgam[pb][:, :], sgam[pb][:, :],
                                    gam_t[0:64, 0:1])
        # s_{t+1} is accumulated on partitions 0-63 but q_{t+1}/x_{t+1}
        # live on 64-127: move it with a tiny sbuf->sbuf DMA
        if os.environ.get("BASS_SWAPDMA", "1") == "1":
            nc.sync.dma_start(sfin[pb][64:128, :], sgam[pb][:, 1:2])
        else:
            nc.vector.tensor_copy(sfin[pb][0:64, :], sgam[pb][:, 1:2])

        for j in range(NBLK):
            # out = (q * (gamma*s)) + x, fused
            nc.vector.scalar_tensor_tensor(
                out=ot_[0:64, j * BN:(j + 1) * BN],
                in0=qs_[0:64, j * BN:(j + 1) * BN],
                scalar=sgam[pb][:, 0:1],
                in1=xp_[0:64, 1 + j * RB:1 + (j + 1) * RB, 1:1 + H].bitcast(F32),
                op0=ALU.mult, op1=ALU.add)
            nc.vector.scalar_tensor_tensor(
                out=ot_[64:128, j * BN:(j + 1) * BN],
                in0=qs_[64:128, j * BN:(j + 1) * BN],
                scalar=sfin[pb][64:128, 0:1],
                in1=xp_[64:128, 1 + j * RB:1 + (j + 1) * RB, 1:1 + H].bitcast(F32),
                op0=ALU.mult, op1=ALU.add)

        nc.gpsimd.dma_start(out_d[:, 2 * p], ot_[0:64, :])
        nc.gpsimd.dma_start(out_d[:, 2 * p + 1], ot_[64:128, :])


_ONES = np.ones((128, BN), np.float32)
_ZER = np.zeros((128, HP), np.float32)

_CACHE = {}


def _build():
    if "nc" in _CACHE:
        return _CACHE["nc"]
    nc = bacc.Bacc("TRN2", target_bir_lowering=False, debug=False,
                   enable_asserts=False, num_devices=8)
    x_d = nc.dram_tensor("x", (C, T, W, H), F32R, kind="ExternalInput").ap()
    wqk_d = nc.dram_tensor("wqk", (128, NTAP, 128), F32R,
                           kind="ExternalInput").ap()
    wv_d = nc.dram_tensor("wv2", (128, NTAP, 128), F32R,
                          kind="ExternalInput").ap()
    gam_d = nc.dram_tensor("gamma_bc", (128, 1), F32,
                           kind="ExternalInput").ap()
    ones_d = nc.dram_tensor("ones", (128, BN), F32R,
                            kind="ExternalInput").ap()
    zer_d = nc.dram_tensor("zer", (128, HP), F32R,
                           kind="ExternalInput").ap()
    out_d = nc.dram_tensor("out", (C, T, W, H), F32,
                           kind="ExternalOutput").ap()
    from contextlib import ExitStack
    with tile.TileContext(nc) as tc, ExitStack() as ctx:
        _emit(nc, tc, x_d, wqk_d, wv_d, gam_d, ones_d, zer_d, out_d, ctx)
    nc.compile()
    _CACHE["nc"] = nc
    return nc


def run_spmd(x, wq, wk, wv, bq, bk, bv, gamma, trace=False, **kw):
    nc = _build()
    wqk, wv2 = _pack_weights(
        np.asarray(wq, np.float32), np.asarray(wk, np.float32),
        np.asarray(wv, np.float32), np.asarray(bq, np.float32),
        np.asarray(bk, np.float32), np.asarray(bv, np.float32))
    gam = np.full((128, 1), np.float32(np.asarray(gamma).reshape(-1)[0]),
                  np.float32)
    x = np.asarray(x, np.float32)
    in_maps = [
        {"x": np.ascontiguousarray(x[b]), "wqk": wqk, "wv2": wv2,
         "gamma_bc": gam, "ones": _ONES, "zer": _ZER}
        for b in range(B)
    ]
    res = bass_utils.run_bass_kernel_spmd(
        nc, in_maps, core_ids=list(range(B)), trace=trace, **kw)
    out = np.stack([res.results[b]["out"] for b in range(B)], axis=0)
    return out, res


def kernel(x, wq, wk, wv, bq, bk, bv, gamma):
    out, _ = run_spmd(x, wq, wk, wv, bq, bk, bv, gamma)
    return out



# revision 5
# speedup vs baseline: 1.6135x; 1.6135x over previous
"""Trainium2 Bass kernel for conv-qkv rank-1 attention (bf16 pipeline).

out = gamma * q * sum(k*v) + x, where q,k,v are per-time-slice 3x3 convs
(C=64 -> C=64) of x [B=8, C=64, T=16, W=64, H=64].

Sharding: data-parallel over B across 8 cores (1 example/core), conv
weights replicated. No cross-core communication.

Per-core schedule: T slices in pairs; slice t on SBUF partitions 0-63,
slice t+1 on 64-127 -> two concurrent PE row-group chains (K=64), which
maxes the array fill rate (1 col/cycle/chain). All matmuls are uniform
64x128 stationaries (geometry changes stall the array ~300ns).
Everything streams bf16, PSUM accumulates f32:
  - x is staged twice (interior at even and odd column offsets) so every
    3x3 tap window is 4B-aligned -- unaligned bf16 moving operands cost
    ~20% fill rate.
  - chain-lo stationary [Wq|Wk] (q_t -> psum parts 0-63, aligned with
    x_t), chain-hi [Wk|Wq]; v uses zero-padded [0|Wv] / [Wv|0] into two
    psum tiles (4 tiles = 8 banks, double buffered).
  - Biases fold into the PSUM->SBUF evictions (Identity activation with
    per-partition bias), which also downcast to bf16. No bias tap.
  - kv mult+pixel-sum: one DVE STT per block half with hw accumulator.
  - out = q*(gamma*s) + x: quarter-slice DVE STTs, bf16 in/out; host
    upcasts. Host-padded x keeps loads as single contiguous DMAs.
"""

import numpy as np
import ml_dtypes

import concourse.bacc as bacc
import concourse.bass as bass
import concourse.mybir as mybir
import concourse.tile as tile
from concourse import bass_utils

F32 = mybir.dt.float32
BF16 = mybir.dt.bfloat16
ALU = mybir.AluOpType
ACT = mybir.ActivationFunctionType
NPBF16 = np.dtype(ml_dtypes.bfloat16)

B, C, T, W, H = 8, 64, 16, 64, 64
WP, HP = W + 2, H + 4            # pad rows [1,65); cols [2,66) / [3,67)
NPAIR = T // 2
RB = 8                           # W-rows per pixel block
NBLK = W // RB
BN = RB * H                      # moving free dim per matmul (512)
NTAP = 9
QC = 2                           # blocks per out-writeback chunk


def _pack_weights(wq, wk, wv):
    def taps(w):  # [O, I, 1, 3, 3] -> [I, 9, O]
        return np.ascontiguousarray(
            np.asarray(w, np.float32).reshape(C, C, 9).transpose(1, 2, 0))

    wq_t, wk_t, wv_t = taps(wq), taps(wk), taps(wv)
    wqk = np.zeros((128, NTAP, 128), np.float32)
    wqk[0:64, :, 0:64] = wq_t
    wqk[0:64, :, 64:128] = wk_t
    wqk[64:128, :, 0:64] = wk_t
    wqk[64:128, :, 64:128] = wq_t
    # v: chain-lo -> psum parts 64-127 (with k_t), chain-hi -> parts 0-63
    wv2 = np.zeros((128, NTAP, 128), np.float32)
    wv2[0:64, :, 64:128] = wv_t
    wv2[64:128, :, 0:64] = wv_t
    return wqk.astype(NPBF16), wv2.astype(NPBF16)


def _emit(nc, tc, xe_d, xo_d, wqk_d, wv_d, gam_d, blo_d, bhi_d, bvv_d,
          out_d, ctx):
    const = ctx.enter_context(tc.tile_pool(name="const", bufs=1))
    state = ctx.enter_context(tc.tile_pool(name="state", bufs=1))
    psum = ctx.enter_context(
        tc.tile_pool(name="psum", bufs=2, space=bass.MemorySpace.PSUM))
    vpool = ctx.enter_context(tc.tile_pool(name="vpool", bufs=3))

    wqk_t = const.tile([128, NTAP, 128], BF16, tag="wqk")
    wv_t = const.tile([128, NTAP, 128], BF16, tag="wv")
    gam_t = const.tile([128, 1], F32, tag="gam")
    blo_t = const.tile([128, 1], F32, tag="blo")
    bhi_t = const.tile([128, 1], F32, tag="bhi")
    bvv_t = const.tile([128, 1], F32, tag="bvv")

    nc.sync.dma_start(wqk_t[:], wqk_d[:])
    nc.sync.dma_start(wv_t[:], wv_d[:])
    nc.sync.dma_start(gam_t[:], gam_d[:])
    nc.sync.dma_start(blo_t[:], blo_d[:])
    nc.sync.dma_start(bhi_t[:], bhi_d[:])
    nc.sync.dma_start(bvv_t[:], bvv_d[:])

    xe = [state.tile([128, WP, HP], BF16, tag=f"xe{i}", name=f"xe{i}")
          for i in range(3)]
    xo = [state.tile([128, WP, HP], BF16, tag=f"xo{i}", name=f"xo{i}")
          for i in range(3)]
    qk_lo = [state.tile([128, NBLK, BN], BF16, tag=f"qlo{i}", name=f"qlo{i}")
             for i in range(2)]
    qk_hi = [state.tile([128, NBLK, BN], BF16, tag=f"qhi{i}", name=f"qhi{i}")
             for i in range(2)]
    ot = [state.tile([128, NBLK, BN], BF16, tag=f"ot{i}", name=f"ot{i}")
          for i in range(2)]
    scr = state.tile([128, BN], BF16, tag="scr")
    sacc = [state.tile([128, NBLK], F32, tag=f"sa{i}", name=f"sa{i}")
            for i in range(2)]
    sful = [state.tile([128, 1], F32, tag=f"sf{i}", name=f"sf{i}")
            for i in range(2)]
    gsw = [state.tile([128, 1], F32, tag=f"gw{i}", name=f"gw{i}")
           for i in range(2)]

    def load_pair(p):
        te, to = xe[p % 3], xo[p % 3]
        nc.sync.dma_start(te[0:64], xe_d[2 * p])
        nc.sync.dma_start(te[64:128], xe_d[2 * p + 1])
        nc.sync.dma_start(to[0:64], xo_d[2 * p])
        nc.sync.dma_start(to[64:128], xo_d[2 * p + 1])

    load_pair(0)
    if NPAIR > 1:
        load_pair(1)

    for p in range(NPAIR):
        pb = p % 2
        xe_, xo_ = xe[p % 3], xo[p % 3]
        qlo_, qhi_, ot_ = qk_lo[pb], qk_hi[pb], ot[pb]

        if p + 2 < NPAIR:
            load_pair(p + 2)

        for j in range(NBLK):
            pqk_lo = psum.tile([128, BN], F32, tag="pqk_lo")
            pqk_hi = psum.tile([128, BN], F32, tag="pqk_hi")
            pvv_lo = psum.tile([128, BN], F32, tag="pvv_lo", name="pvv_lo")
            pvv_hi = psum.tile([128, BN], F32, tag="pvv_hi", name="pvv_hi")

            def rhs(half, tap):
                dy, dx = tap // 3, tap % 3
                r0 = j * RB + dy
                base = 64 * half
                if dx == 1:
                    return xe_[base:base + 64, r0:r0 + RB, 2:2 + H]
                if dx == 0:
                    return xo_[base:base + 64, r0:r0 + RB, 2:2 + H]
                return xo_[base:base + 64, r0:r0 + RB, 4:4 + H]

            for tap in range(NTAP):
                st, sp = tap == 0, tap == NTAP - 1
                nc.tensor.matmul(pqk_lo[:, :], wqk_t[0:64, tap, :],
                                 rhs(0, tap), start=st, stop=sp)
                nc.tensor.matmul(pqk_hi[:, :], wqk_t[64:128, tap, :],
                                 rhs(1, tap), start=st, stop=sp)
            for tap in range(NTAP):
                st, sp = tap == 0, tap == NTAP - 1
                nc.tensor.matmul(pvv_lo[:, :], wv_t[0:64, tap, :],
                                 rhs(0, tap), start=st, stop=sp)
                nc.tensor.matmul(pvv_hi[:, :], wv_t[64:128, tap, :],
                                 rhs(1, tap), start=st, stop=sp)

            # evict psum -> bf16 sbuf, adding conv biases (per-partition)
            nc.scalar.activation(qlo_[:, j, :], pqk_lo[:, :], ACT.Identity,
                                 bias=blo_t[:, 0:1])
            nc.scalar.activation(qhi_[:, j, :], pqk_hi[:, :], ACT.Identity,
                                 bias=bhi_t[:, 0:1])
            vsb = vpool.tile([128, BN], BF16, tag="vsb", name="vsb")
            nc.scalar.activation(vsb[64:128, :], pvv_lo[64:128, :],
                                 ACT.Identity, bias=bvv_t[64:128, 0:1])
            nc.scalar.activation(vsb[0:64, :], pvv_hi[0:64, :],
                                 ACT.Identity, bias=bvv_t[0:64, 0:1])

            # fused k*v multiply + pixel-sum
            nc.vector.scalar_tensor_tensor(
                out=scr[64:128, :], in0=qlo_[64:128, j, :], scalar=1.0,
                in1=vsb[64:128, :], op0=ALU.mult, op1=ALU.mult,
                accum_out=sacc[pb][64:128, j:j + 1])
            nc.vector.scalar_tensor_tensor(
                out=scr[0:64, :], in0=qhi_[0:64, j, :], scalar=1.0,
                in1=vsb[0:64, :], op0=ALU.mult, op1=ALU.mult,
                accum_out=sacc[pb][0:64, j:j + 1])

        nc.vector.reduce_sum(sful[pb][:, :], sacc[pb][:, :],
                             axis=mybir.AxisListType.X)
        nc.vector.tensor_scalar_mul(sful[pb][:, :], sful[pb][:, :],
                                    gam_t[:, 0:1])
        # gs accumulated on k's partitions = complement of q's: swap halves
        nc.scalar.copy(gsw[pb][0:64, :], sful[pb][64:128, :])
        nc.scalar.copy(gsw[pb][64:128, :], sful[pb][0:64, :])

        for m in range(0, NBLK, QC):
            # out = q * (gamma*s) + x, fused (bf16), QC blocks per op
            r0 = 1 + m * RB
            nc.vector.scalar_tensor_tensor(
                out=ot_[0:64, m:m + QC, :],
                in0=qlo_[0:64, m:m + QC, :],
                scalar=gsw[pb][0:64, 0:1],
                in1=xe_[0:64, r0:r0 + QC * RB, 2:2 + H],
                op0=ALU.mult, op1=ALU.add)
            nc.vector.scalar_tensor_tensor(
                out=ot_[64:128, m:m + QC, :],
                in0=qhi_[64:128, m:m + QC, :],
                scalar=gsw[pb][64:128, 0:1],
                in1=xe_[64:128, r0:r0 + QC * RB, 2:2 + H],
                op0=ALU.mult, op1=ALU.add)
            nc.gpsimd.dma_start(out_d[2 * p, :, m * RB * H:(m + QC) * RB * H],
                                ot_[0:64, m:m + QC, :])
            nc.gpsimd.dma_start(
                out_d[2 * p + 1, :, m * RB * H:(m + QC) * RB * H],
                ot_[64:128, m:m + QC, :])


_CACHE = {}


def _build():
    if "nc" in _CACHE:
        return _CACHE["nc"]
    nc = bacc.Bacc("TRN2", target_bir_lowering=False, debug=False,
                   enable_asserts=False, num_devices=8)
    xe_d = nc.dram_tensor("xe16", (T, C, WP, HP), BF16,
                          kind="ExternalInput").ap()
    xo_d = nc.dram_tensor("xo16", (T, C, WP, HP), BF16,
                          kind="ExternalInput").ap()
    wqk_d = nc.dram_tensor("wqk", (128, NTAP, 128), BF16,
                           kind="ExternalInput").ap()
    wv_d = nc.dram_tensor("wv2", (128, NTAP, 128), BF16,
                          kind="ExternalInput").ap()
    gam_d = nc.dram_tensor("gamma_bc", (128, 1), F32,
                           kind="ExternalInput").ap()
    blo_d = nc.dram_tensor("b_lo", (128, 1), F32, kind="ExternalInput").ap()
    bhi_d = nc.dram_tensor("b_hi", (128, 1), F32, kind="ExternalInput").ap()
    bvv_d = nc.dram_tensor("b_vv", (128, 1), F32, kind="ExternalInput").ap()
    out_d = nc.dram_tensor("out", (T, C, W * H), BF16,
                           kind="ExternalOutput").ap()
    from contextlib import ExitStack
    with tile.TileContext(nc) as tc, ExitStack() as ctx:
        _emit(nc, tc, xe_d, xo_d, wqk_d, wv_d, gam_d, blo_d, bhi_d, bvv_d,
              out_d, ctx)
    nc.compile()
    _CACHE["nc"] = nc
    return nc


def run_spmd(x, wq, wk, wv, bq, bk, bv, gamma, trace=False, **kw):
    nc = _build()
    wqk, wv2 = _pack_weights(wq, wk, wv)
    bq = np.asarray(bq, np.float32).reshape(C)
    bk = np.asarray(bk, np.float32).reshape(C)
    bv = np.asarray(bv, np.float32).reshape(C)
    blo = np.concatenate([bq, bk]).reshape(128, 1)
    bhi = np.concatenate([bk, bq]).reshape(128, 1)
    bvv = np.concatenate([bv, bv]).reshape(128, 1)
    gam = np.full((128, 1), np.float32(np.asarray(gamma).reshape(-1)[0]),
                  np.float32)
    x = np.asarray(x, np.float32)
    in_maps = []
    for b in range(B):
        xt = x[b].transpose(1, 0, 2, 3).astype(NPBF16)
        xe = np.zeros((T, C, WP, HP), NPBF16)
        xe[:, :, 1:1 + W, 2:2 + H] = xt
        xo = np.zeros((T, C, WP, HP), NPBF16)
        xo[:, :, 1:1 + W, 3:3 + H] = xt
        in_maps.append({"xe16": xe, "xo16": xo, "wqk": wqk, "wv2": wv2,
                        "gamma_bc": gam, "b_lo": blo, "b_hi": bhi,
                        "b_vv": bvv})
    res = bass_utils.run_bass_kernel_spmd(
        nc, in_maps, core_ids=list(range(B)), trace=trace, **kw)
    out = np.stack(
        [res.results[b]["out"].astype(np.float32)
         .reshape(T, C, W, H).transpose(1, 0, 2, 3) for b in range(B)],
        axis=0)
    return out, res


def kernel(x, wq, wk, wv, bq, bk, bv, gamma):
    out, _ = run_spmd(x, wq, wk, wv, bq, bk, bv, gamma)
    return out
